# revision 22
# baseline (speedup 1.0000x reference)
"""Sparse (causal + kv-padding) attention on 8 TRN2 NeuronCores via Bass/Tile.

Shapes (hardcoded per spec): B=2, H=16, S=2048, D=64, fp32.
Sharding: batch*head (32 pairs) split 4-per-core across 8 cores; no collectives.

Design (v3 structure + input prefetch):
  S^T[kv, q] = K @ Q^T            (TensorE, contraction d=64, kv tiles row-paired)
  P^T = exp(S^T * scale)          split across two engines per a static table:
    ACT path:  scalar.activation(Exp, scale) PSUM->SBUF fp16 (exact)
    DVE path:  Schraudolph bit-trick: tensor_scalar(out_int16, S, A16, B16,
               mult, add) — round-half-even + saturation makes
               bitcast-as-fp16(round(S*A16+B16)) ~= exp(S*scale) (~3% elem err).
               For diagonal tiles the causal mask folds into a per-element bias
               operand (scalar_tensor_tensor): masked lanes saturate to -32768
               = 0x8000 = fp16 -0.0, which the PV matmul treats as zero.
               (CoreSim models the int16 convert as wraparound, so this kernel
               is sim-incorrect at masked positions; HW saturates — verified.)
    qb0's diagonal tiles (q<512 rows with few kv entries — the accuracy-
    critical ones) always use the exact ACT path + DVE 0/1 tri-mask multiply.
  kv padding: folded into V_aug = [V*kvmask | kvmask] host-side.
  O^T_aug[65, q] = V_aug^T @ P^T  (TensorE, PSUM-accumulated; row 64 = sum P)
  O^T_aug evacuated PSUM->SBUF (split ACT/DVE) and DMA'd out UNNORMALIZED.
  Host: O = (O^T[0:64] / O^T[64]).T  — normalization + transpose on host.

  PE stream is software-pipelined three groups deep so the in-order PE
  does not wait on the exp of the group it just produced; next head's input
  DMAs are emitted before this head's compute so the in-order sync queue
  never blocks them behind output DMAs.
"""

import math
import os
import time
from contextlib import ExitStack

import numpy as np

import concourse.bass as bass
import concourse.mybir as mybir
import concourse.tile as tile
from concourse import bacc
from concourse.bass_utils import run_bass_kernel_spmd

B, H, S, D = 2, 16, 2048, 64
N_CORES = 8
HPC = (B * H) // N_CORES  # heads per core = 4
NKV = S // 128            # 16 kv tiles per head
QB = 512                  # q block width (PSUM bank)
NQB = S // QB             # 4 q blocks
SCALE = 1.0 / math.sqrt(D)
F32 = mybir.dt.float32
F16 = mybir.dt.float16
I16 = mybir.dt.int16

# Schraudolph constants for fp16-bits exp: bits = round(S_raw * A16 + B16)
A16 = float(np.float32(1024.0 * np.log2(np.e) * SCALE))
B16 = 15360.0           # 15 * 1024 (fp16 exponent bias << mantissa bits)
NEGBIG = -1.0e9         # bias for masked lanes -> saturates to 0x8000 = -0.0

# Diagonal group packing in a 3-bank (1536-col) PSUM tile:
# tile t (kv tile diag0+t) at psum col DIAG_PCOL[t], width DIAG_W[t], q off 128t
DIAG_PCOL = [0, 512, 1024, 896]
DIAG_W = [512, 384, 256, 128]

last_results = None


def _head_groups():
    """Flat per-head group list in emission order.

    Each group: dict(qb, kind, eng, items=[(j, pcol, w, qoff), ...], last_of_qb)
    Emission order interleaves ACT/DVE so neither exp engine queue backs up;
    within a qb the first-emitted group always contains tile j covering the
    full q range (for the PSUM start=True flag).
    """
    groups = []
    for qb in range(NQB):
        diag0 = 4 * qb
        diag_items = [(diag0 + t, DIAG_PCOL[t], DIAG_W[t], 128 * t)
                      for t in range(4)]

        def full(a, b):
            return [(j, 512 * (j - a), 512, 0) for j in range(a, b)]

        if qb == 0:
            order = [("diag", "act", diag_items)]
        elif qb == 1:
            order = [("full", "dve", full(0, 2)),
                     ("full", "act", full(2, 4)),
                     ("diag", "dve", diag_items)]
        elif qb == 2:
            order = [("full", "act", full(0, 3)),
                     ("full", "dve", full(6, 8)),
                     ("full", "act", full(3, 6)),
                     ("diag", "dve", diag_items)]
        else:
            order = [("full", "act", full(0, 3)),
                     ("full", "dve", full(9, 12)),
                     ("full", "act", full(3, 6)),
                     ("diag", "dve", diag_items),
                     ("full", "act", full(6, 9))]
        for i, (kind, eng, items) in enumerate(order):
            groups.append(dict(qb=qb, kind=kind, eng=eng, items=items,
                               last_of_qb=(i == len(order) - 1)))
    return groups


def _build_program():
    nc = bacc.Bacc("TRN2", target_bir_lowering=False, debug=False,
                   num_devices=N_CORES)
    qt_d = nc.dram_tensor("qt", [HPC, 128, S], F16, kind="ExternalInput")
    kt_d = nc.dram_tensor("kt", [HPC, 128, NKV // 2, 128], F16,
                          kind="ExternalInput")
    va_d = nc.dram_tensor("va", [HPC, 128, NKV, 128], F16,
                          kind="ExternalInput")
    btri_d = nc.dram_tensor("btri", [128, 1536], F32, kind="ExternalInput")
    utm_d = nc.dram_tensor("utm", [128, 1536], F16, kind="ExternalInput")
    out_d = nc.dram_tensor("out", [HPC, 65, S], F32, kind="ExternalOutput")

    groups = _head_groups()
    evac_eng = {0: "act", 1: "dve", 2: "dve", 3: "dve"}

    with ExitStack() as ctx:
        tc = ctx.enter_context(tile.TileContext(nc))
        const_pool = ctx.enter_context(tc.tile_pool(name="const", bufs=1))
        qt_pool = ctx.enter_context(tc.tile_pool(name="qtp", bufs=2))
        kt_pool = ctx.enter_context(tc.tile_pool(name="ktp", bufs=2))
        va_pool = ctx.enter_context(tc.tile_pool(name="vap", bufs=2))
        pt_pool = ctx.enter_context(tc.tile_pool(name="ptp", bufs=6))
        outsb_pool = ctx.enter_context(tc.tile_pool(name="osp", bufs=2))
        sps_pool = ctx.enter_context(tc.tile_pool(name="sps", bufs=2,
                                                  space="PSUM"))
        oacc_pool = ctx.enter_context(tc.tile_pool(name="oac", bufs=2,
                                                   space="PSUM"))

        def load_head(hl):
            qt = qt_pool.tile([128, S], F16, tag="qt", name="qt")
            kt = kt_pool.tile([128, NKV // 2, 128], F16, tag="kt", name="kt")
            nc.sync.dma_start(qt[:, 0:QB], qt_d[hl, :, 0:QB])
            nc.sync.dma_start(kt[:, 0:2, :], kt_d[hl, :, 0:2, :])
            nc.sync.dma_start(qt[:, QB:S], qt_d[hl, :, QB:S])
            nc.sync.dma_start(kt[:, 2:, :], kt_d[hl, :, 2:, :])
            va = va_pool.tile([128, NKV, 128], F16, tag="va", name="va")
            nc.sync.dma_start(va[:, 0:4, :], va_d[hl, :, 0:4, :])
            nc.sync.dma_start(va[:, 4:, :], va_d[hl, :, 4:, :])
            return qt, kt, va

        tiles = load_head(0)
        btri = const_pool.tile([128, 1536], F32)
        nc.sync.dma_start(btri[:, :], btri_d[:, :])
        utm = const_pool.tile([128, 1536], F16)
        nc.sync.dma_start(utm[:, :], utm_d[:, :])
        for hl in range(HPC):
            qt, kt, va = tiles
            if hl + 1 < HPC:
                tiles = load_head(hl + 1)

            outsb = outsb_pool.tile([65, S], F32, tag="outsb", name="outsb")
            oaccs = {}
            pend = []
            first_pv_done = set()

            def emit_pv(g, pt, oacc, hl=hl, va=va, outsb=outsb):
                qb = g["qb"]
                q0 = qb * QB
                start = qb not in first_pv_done
                if start:
                    assert g["items"][0][3] == 0 and g["items"][0][2] == QB
                    first_pv_done.add(qb)
                n = len(g["items"])
                for i, (j, pcol, w, qoff) in enumerate(g["items"]):
                    nc.tensor.matmul(
                        oacc[:, qoff:QB],
                        va[:, j, :],
                        pt[:, pcol:pcol + w],
                        start=(start and i == 0),
                        stop=(g["last_of_qb"] and i == n - 1),
                    )
                if g["last_of_qb"]:
                    if evac_eng[qb] == "act":
                        nc.scalar.copy(outsb[:, q0:q0 + QB], oacc[0:65, :])
                    else:
                        nc.vector.tensor_copy(outsb[:, q0:q0 + QB],
                                              oacc[0:65, :])
                    nc.sync.dma_start(out_d[hl, :, q0:q0 + QB],
                                      outsb[:, q0:q0 + QB])

            for g in groups:
                qb = g["qb"]
                q0 = qb * QB
                if qb not in oaccs:
                    oacc_t = oacc_pool.tile([128, QB], F32, tag="oacc",
                                            name="oacc")
                    oaccs[qb] = oacc_t
                s_ps = sps_pool.tile([128, 1536], F32, tag="sps", name="sps")
                width = max(c + w for _, c, w, _ in g["items"])
                for j, pcol, w, qoff in g["items"]:
                    lo, hi = (0, 64) if j % 2 == 0 else (64, 128)
                    nc.tensor.matmul(
                        s_ps[:, pcol:pcol + w],
                        kt[lo:hi, j // 2, :],
                        qt[lo:hi, q0 + qoff:q0 + QB],
                        start=True, stop=True,
                    )
                pt = pt_pool.tile([128, 1536], F16, tag="pt", name="pt")
                if g["eng"] == "act":
                    nc.scalar.activation(pt[:, :width], s_ps[:, :width],
                                         mybir.ActivationFunctionType.Exp,
                                         scale=SCALE)
                    if g["kind"] == "diag":
                        nc.vector.tensor_mul(pt[:, :width], pt[:, :width],
                                             utm[:, :width])
                else:
                    if g["kind"] == "diag":
                        nc.vector.scalar_tensor_tensor(
                            pt.bitcast(I16)[:, :width], s_ps[:, :width],
                            A16, btri[:, :width],
                            mybir.AluOpType.mult, mybir.AluOpType.add)
                    else:
                        nc.vector.tensor_scalar(
                            pt.bitcast(I16)[:, :width], s_ps[:, :width],
                            A16, B16,
                            mybir.AluOpType.mult, mybir.AluOpType.add)
                pend.append((g, pt, oaccs[qb]))
                if len(pend) > 3:
                    emit_pv(*pend.pop(0))
            for p in pend:
                emit_pv(*p)
    nc.compile()
    return nc


_program_cache = None


def _get_program():
    global _program_cache
    if _program_cache is None:
        _program_cache = _build_program()
    return _program_cache


def _make_consts():
    btri = np.full((128, 1536), NEGBIG, dtype=np.float32)
    utm = np.zeros((128, 1536), dtype=np.float16)
    rr = np.arange(128)[:, None]
    for t in range(4):
        c0, w = DIAG_PCOL[t], DIAG_W[t]
        qq = np.arange(w)[None, :]
        keep = rr <= qq
        btri[:, c0:c0 + w] = np.where(keep, np.float32(B16),
                                      np.float32(NEGBIG))
        utm[:, c0:c0 + w] = keep.astype(np.float16)
    return btri, utm


def kernel(**inputs):
    q = np.asarray(inputs["query_states"], dtype=np.float32)
    k = np.asarray(inputs["key_states"], dtype=np.float32)
    v = np.asarray(inputs["value_states"], dtype=np.float32)
    kvm = np.asarray(inputs["kv_sequence_mask"])

    qf = q.reshape(B * H, S, D)
    kf = k.reshape(B * H, S, D)
    vf = v.reshape(B * H, S, D)
    btri, utm = _make_consts()

    in_maps = []
    for c in range(N_CORES):
        hs = slice(c * HPC, (c + 1) * HPC)
        b = (c * HPC) // H  # all heads of a core belong to one batch elem

        qt_c = qf[hs].transpose(0, 2, 1)                   # [4, 64, 2048]
        qt_c = np.concatenate([qt_c, qt_c], axis=1)        # [4, 128, 2048]

        kt_t = kf[hs].transpose(0, 2, 1).reshape(HPC, 64, NKV, 128)
        kt_c = np.concatenate([kt_t[:, :, 0::2, :],
                               kt_t[:, :, 1::2, :]], axis=1)  # [4,128,8,128]

        bmask = kvm[b].astype(np.float32)                  # [S]
        va_c = np.zeros((HPC, S, 128), dtype=np.float32)
        va_c[:, :, :D] = vf[hs] * bmask[None, :, None]
        va_c[:, :, D] = bmask[None, :]
        va_c = va_c.reshape(HPC, NKV, 128, 128).transpose(0, 2, 1, 3)

        in_maps.append({
            "qt": np.ascontiguousarray(qt_c).astype(np.float16),
            "kt": np.ascontiguousarray(kt_c).astype(np.float16),
            "va": np.ascontiguousarray(va_c).astype(np.float16),
            "btri": btri,
            "utm": utm,
        })

    nc = _get_program()
    trace = bool(int(os.environ.get("ATTN_TRACE", "0")))
    # The axon-tunneled devices occasionally fail/corrupt the first execution
    # of a freshly loaded NEFF; retry on exceptions and on NaN output.
    last_err = None
    res = None
    for attempt in range(3):
        try:
            res = run_bass_kernel_spmd(nc, in_maps,
                                       core_ids=list(range(N_CORES)),
                                       trace=trace)
            outs = np.stack([r["out"] for r in res.results])  # [8, 4, 65, S]
            if np.isnan(outs).any():
                raise RuntimeError("NaN in device output (transient glitch)")
            break
        except Exception as e:
            last_err = e
            res = None
            time.sleep(15 * (attempt + 1))
    if res is None:
        raise last_err
    global last_results
    last_results = res

    # host-side normalization + transpose
    outs = outs.reshape(B * H, 65, S)
    attn = (outs[:, :D, :] / outs[:, D:D + 1, :]).transpose(0, 2, 1)
    attn = np.ascontiguousarray(attn).reshape(B, H, S, D)
    return (attn, np.asarray(inputs["key_states"]),
            np.asarray(inputs["value_states"]))
